# revision 1
# baseline (speedup 1.0000x reference)
"""Trainium2 Bass kernel for nn_CNNEmbedder (surface-code CNN embedder).

Math: per (batch, window) the int recurrence produces st in {-1,0,1} per
ancilla; output col p (pair (i,j)) is a per-pair 6-value table lookup
T_p[d_i, d_j] (d = 0 for st=+1, 1 for st=0, 2 for st=-1).

Device scheme (per 128-batch tile x window):
  T_p[d_i,d_j] = U*V + W, with U/V/W each "outer-sum" planes
      U[b,p] = xU[p, d_i] + yU[p, d_j]  (same for V, W)
  computed by ONE K=97 matmul each (K-rows = one-hot st encodings e0,e1
  per ancilla + const row).  Host precomputes the per-pair tables
  (closed-form linear solve).  DVE writes U*V into PSUM, PE accumulates
  the W matmul on top (start=False), DMA streams PSUM -> DRAM.

Sharding: pure batch data-parallel across 8 cores (512 batch each).
"""
import sys

sys.path.insert(0, "/opt/trn_rl_repo")

import numpy as np
import ml_dtypes
from contextlib import ExitStack

import concourse.bass as bass
import concourse.tile as tile
from concourse import bacc
from concourse import mybir
from concourse import bass_utils
from concourse.masks import make_identity

F32 = mybir.dt.float32
F32R = mybir.dt.float32r
BF16 = mybir.dt.bfloat16
AL = mybir.AluOpType

A = 48            # ancillas
R = 25            # rounds
NW = 23           # windows (R-2)
ND = 1176         # output cols (48 diag + 1128 nondiag)
NPAIR = 1128
TH = 392          # third of ND, fits one PSUM bank (392*4B = 1568 <= 2048)
P = 128
NBT = 4           # batch tiles per core (512 = 4*128)
BCORE = 512       # batch per core
K = 97            # matmul contraction rows: 48*e0 + 48*e1 + const

_PROGRAM_CACHE = {}


# ---------------------------------------------------------------- host math
def _pair_list():
    pairs = []
    for iy in range(A):
        for ix in range(iy + 1, A):
            pairs.append((iy, ix))
    return pairs


def _decompose(T64):
    """T64 (N,3,3) -> tables xU,yU,xV,yV,xW,yW each (N,3) f64 with
    T = (xU(+)yU) * (xV(+)yV) + (xW(+)yW)."""
    N = T64.shape[0]
    D = (T64[:, 0:2, 0:2] - T64[:, 0:2, 2:3] - T64[:, 2:3, 0:2]
         + T64[:, 2:3, 2:3])

    def build(D, swap):
        Dl = np.swapaxes(D, 1, 2) if swap else D
        Y0 = Dl[:, 0, 0] - Dl[:, 1, 0]
        Y1 = Dl[:, 0, 1] - Dl[:, 1, 1]
        s = np.maximum(np.sqrt(Y0**2 + Y1**2), 1e-300)
        Y0n, Y1n = Y0 / s, Y1 / s
        Rr = Dl[:, 0, 1] * Y0n - Dl[:, 0, 0] * Y1n
        d0 = -Rr * Y1n
        d1 = Rr * Y0n
        use0 = np.abs(Y0n) >= np.abs(Y1n)
        g0 = np.where(use0, (Dl[:, 0, 0] - d0) / np.where(use0, Y0n, 1.0),
                      (Dl[:, 0, 1] - d1) / np.where(~use0, Y1n, 1.0))
        g1 = np.where(use0, (Dl[:, 1, 0] - d0) / np.where(use0, Y0n, 1.0),
                      (Dl[:, 1, 1] - d1) / np.where(~use0, Y1n, 1.0))
        one = np.ones(N)
        zer = np.zeros(N)
        x = np.stack([one, one, zer], -1)
        y = np.stack([Y0n, Y1n, zer], -1)
        g = np.stack([g0, g1, zer], -1)
        d = np.stack([d0, d1, zer], -1)
        if swap:
            return y, x, d, g
        return x, y, g, d

    xa, ya, ga, da = build(D, False)
    xb, yb, gb, db = build(D, True)
    conda = np.max(np.abs(np.concatenate([ga, da], -1)), -1)
    condb = np.max(np.abs(np.concatenate([gb, db], -1)), -1)
    pa = (conda <= condb)[:, None]
    xU = np.where(pa, xa, xb)
    yU = np.where(pa, ya, yb)
    xV = np.where(pa, ga, gb)
    yV = np.where(pa, da, db)
    U = xU[:, :, None] + yU[:, None, :]
    V = xV[:, :, None] + yV[:, None, :]
    W = T64 - U * V
    xW = W[:, :, 2] - W[:, 2:3, 2]          # phi_d  (W22 folded)
    yW = W[:, 2, :]                          # psi_d'
    return xU, yU, xV, yV, xW, yW


def _host_tables(emb_diag, emb_nondiag):
    """Build rhs tables ru, rv, rw: (K, ND) f32."""
    sig_diag = 1.0 / (1.0 + np.exp(-emb_diag[0].astype(np.float64)))   # (48,)
    sg = 1.0 / (1.0 + np.exp(-emb_nondiag[0].astype(np.float64)))      # (1128,4)
    P1 = sg[:, 0]
    P2 = sg[:, 1] * P1
    P3 = sg[:, 2] * P2
    P4 = sg[:, 3] * P3
    N = NPAIR
    T = np.zeros((N, 3, 3))
    T[:, 0, 0] = 1.0
    T[:, 0, 1] = P1; T[:, 1, 0] = P1
    T[:, 1, 1] = P2
    T[:, 0, 2] = P3; T[:, 2, 0] = P3
    T[:, 1, 2] = P4; T[:, 2, 1] = P4
    xU, yU, xV, yV, xW, yW = _decompose(T)

    ru = np.zeros((K, ND))
    rv = np.zeros((K, ND))
    rw = np.zeros((K, ND))
    # diag columns 0..47: value = W only: d=0 -> 1, d=1 -> sig_diag, d=2 -> 0
    for a in range(A):
        rw[0 * A + a, a] = 1.0            # e0 coeff (D[a,0]-D[a,2])
        rw[1 * A + a, a] = sig_diag[a]    # e1 coeff
    # nondiag
    pairs = _pair_list()
    for q, (i, j) in enumerate(pairs):
        col = A + q
        for tabs, rmat in ((( xU, yU), ru), ((xV, yV), rv), ((xW, yW), rw)):
            xt, yt = tabs
            for m in (0, 1):
                rmat[m * A + i, col] += xt[q, m] - xt[q, 2]
                rmat[m * A + j, col] += yt[q, m] - yt[q, 2]
            rmat[K - 1, col] += xt[q, 2] + yt[q, 2]

    def split_hi_lo(t64):
        hi = t64.astype(np.float32)
        # truncate mantissa to 10 bits: exact on any f32r grid (>=10 bits)
        bits = hi.view(np.int32)
        bits &= np.int32(~((1 << 13) - 1))
        hi = bits.view(np.float32)
        lo = (t64 - hi.astype(np.float64)).astype(np.float32)
        return hi, lo

    ru_hi, ru_lo = split_hi_lo(ru)
    rv_hi, rv_lo = split_hi_lo(rv)
    rw_hi, rw_lo = split_hi_lo(rw)
    return ru_hi, ru_lo, rv_hi, rv_lo, rw_hi, rw_lo


# ---------------------------------------------------------------- program
def _build_program():
    nc = bacc.Bacc(None, target_bir_lowering=False)
    xs_d = nc.declare_dram_parameter("xs", [BCORE, R * A], BF16, isOutput=False)
    rt_d = {}
    for nm in ("ru_hi", "ru_lo", "rv_hi", "rv_lo", "rw_hi", "rw_lo"):
        rt_d[nm] = nc.declare_dram_parameter(nm, [K, ND], F32R, isOutput=False)
    out_d = nc.declare_dram_parameter("out", [BCORE, NW, ND], F32, isOutput=True)

    WIDE = NW * A  # 1104

    with ExitStack() as ctx:
        tc = ctx.enter_context(tile.TileContext(nc))
        singles = ctx.enter_context(tc.tile_pool(name="singles", bufs=1))
        wscr = ctx.enter_context(tc.tile_pool(name="wscr", bufs=4))
        sscr = ctx.enter_context(tc.tile_pool(name="sscr", bufs=4))
        epool = ctx.enter_context(tc.tile_pool(name="epool", bufs=3))
        lhp = ctx.enter_context(tc.tile_pool(name="lhp", bufs=3))
        vsp = ctx.enter_context(tc.tile_pool(name="vsp", bufs=6))
        outp = ctx.enter_context(tc.tile_pool(name="outp", bufs=4))
        pT = ctx.enter_context(tc.tile_pool(name="pT", bufs=1, space="PSUM"))
        pUV = ctx.enter_context(tc.tile_pool(name="pUV", bufs=3, space="PSUM"))

        ident = singles.tile([P, P], F32)
        make_identity(nc, ident)
        rt_s = {}
        for nm in ("ru_hi", "ru_lo", "rv_hi", "rv_lo", "rw_hi", "rw_lo"):
            rt_s[nm] = singles.tile([K, ND], F32R, tag=nm, name=nm + "_s")
            nc.sync.dma_start(out=rt_s[nm], in_=rt_d[nm][:, :])
        identr = singles.tile([P, P], F32R, tag="identr")
        nc.vector.tensor_copy(identr, ident)

        xts = []
        for bt in range(NBT):
            xt = singles.tile([P, R * A], BF16, tag=f"x{bt}")
            nc.sync.dma_start(out=xt, in_=xs_d[bt * P:(bt + 1) * P, :])
            xts.append(xt)

        de_t = singles.tile([P, NBT, WIDE], BF16, tag="de")
        me2_t = singles.tile([P, NBT, WIDE], BF16, tag="me2")
        mep_t = singles.tile([P, NBT, WIDE], BF16, tag="mep")
        one_t = singles.tile([P, WIDE], BF16, tag="one")
        nc.gpsimd.memset(one_t, 1.0)

        # ---- wide precompute (GPSIMD): per b-tile
        for bt in range(NBT):
            xt = xts[bt]
            a_ap = xt[:, 0:WIDE]
            b_ap = xt[:, A:A + WIDE]
            c_ap = xt[:, 2 * A:2 * A + WIDE]
            t1 = wscr.tile([P, WIDE], BF16, tag="w0")
            d0 = wscr.tile([P, WIDE], BF16, tag="w1")
            w1 = wscr.tile([P, WIDE], BF16, tag="w2")
            u1 = wscr.tile([P, WIDE], BF16, tag="w3")
            u2 = wscr.tile([P, WIDE], BF16, tag="w4")
            nme = wscr.tile([P, WIDE], BF16, tag="w5")
            tmp = wscr.tile([P, WIDE], BF16, tag="w6")
            de1 = wscr.tile([P, WIDE], BF16, tag="w7")
            g = nc.gpsimd
            g.tensor_tensor(t1, a_ap, c_ap, AL.mult)
            g.tensor_tensor(d0, a_ap, c_ap, AL.subtract)
            g.tensor_tensor(de_t[:, bt, :], d0, d0, AL.mult)
            g.tensor_tensor(w1, b_ap, t1, AL.mult)
            g.tensor_tensor(u1, b_ap, t1, AL.add)
            # u2 = u1 - 2*w1
            g.tensor_tensor(tmp, w1, w1, AL.add)
            g.tensor_tensor(u2, u1, tmp, AL.subtract)
            # nme = (de - 1) * u2   ( = -meas_err )
            g.tensor_tensor(de1, de_t[:, bt, :], one_t, AL.subtract)
            g.tensor_tensor(nme, de1, u2, AL.mult)
            # me2 = 1 - 2*me = 2*nme + 1 ; mep = 1 - me = nme + 1
            g.tensor_tensor(tmp, nme, nme, AL.add)
            g.tensor_tensor(me2_t[:, bt, :], tmp, one_t, AL.add)
            g.tensor_tensor(mep_t[:, bt, :], nme, one_t, AL.add)

        st_t = singles.tile([P, NBT, A], BF16, tag="st")
        dt_t = singles.tile([P, NBT, A], BF16, tag="dt")
        nc.vector.memset(st_t, -1.0)
        nc.vector.memset(dt_t, 1.0)

        ncp = 0  # copy-op round robin counter for C_P balancing
        for w in range(NW):
            de_w = de_t[:, :, w * A:(w + 1) * A]
            me2_w = me2_t[:, :, w * A:(w + 1) * A]
            mep_w = mep_t[:, :, w * A:(w + 1) * A]
            g = nc.gpsimd
            dt1 = sscr.tile([P, NBT, A], BF16, tag="s0")
            q = sscr.tile([P, NBT, A], BF16, tag="s1")
            s = sscr.tile([P, NBT, A], BF16, tag="s2")
            u2s = sscr.tile([P, NBT, A], BF16, tag="s3")
            wv = sscr.tile([P, NBT, A], BF16, tag="s4")
            z = sscr.tile([P, NBT, A], BF16, tag="s5")
            g.tensor_tensor(dt1, dt_t, me2_w, AL.mult)
            g.tensor_tensor(q, dt1, de_w, AL.mult)
            g.tensor_tensor(s, st_t, q, AL.add)
            nc.vector.tensor_scalar(st_t, s, -1.0, 1.0, AL.max, AL.min)
            g.tensor_tensor(u2s, mep_w, st_t, AL.mult)
            g.tensor_tensor(wv, st_t, dt1, AL.mult)
            nc.vector.scalar_tensor_tensor(z, wv, 1.0, u2s, AL.add, AL.mult)
            g.tensor_tensor(dt_t, dt1, z, AL.subtract)

            et = epool.tile([P, NBT, K], F32R, tag="e")
            nc.vector.tensor_scalar(et[:, :, 0:A], st_t, 1.0, None, AL.is_equal)
            nc.vector.tensor_scalar(et[:, :, A:2 * A], st_t, 0.0, None,
                                    AL.is_equal)
            nc.vector.tensor_scalar(et[:, :, 2 * A:K], st_t[:, :, 0:1],
                                    -10.0, None, AL.is_ge)

            pt = pT.tile([K, NBT * P], F32R)
            for bt in range(NBT):
                nc.tensor.transpose(pt[:, bt * P:(bt + 1) * P],
                                    et[:, bt, :], identr)
            lh = lhp.tile([K, NBT * P], F32R, tag="lh")
            nc.scalar.copy(lh, pt)

            for bt in range(NBT):
                lhs_bt = lh[:, bt * P:(bt + 1) * P]
                ot = outp.tile([P, ND], F32, tag="ot")
                for c in range(3):
                    c0 = c * TH
                    ut = pUV.tile([P, TH], F32, tag="u")
                    vt = pUV.tile([P, TH], F32, tag="vv")
                    nc.tensor.matmul(ut, lhs_bt, rt_s["ru_hi"][:, c0:c0 + TH],
                                     start=True, stop=False,
                                     skip_group_check=True)
                    nc.tensor.matmul(ut, lhs_bt, rt_s["ru_lo"][:, c0:c0 + TH],
                                     start=False, stop=True,
                                     skip_group_check=True)
                    nc.tensor.matmul(vt, lhs_bt, rt_s["rv_hi"][:, c0:c0 + TH],
                                     start=True, stop=False,
                                     skip_group_check=True)
                    nc.tensor.matmul(vt, lhs_bt, rt_s["rv_lo"][:, c0:c0 + TH],
                                     start=False, stop=True,
                                     skip_group_check=True)
                    vs = vsp.tile([P, TH], F32, tag="vs")
                    lo = A if c == 0 else 0
                    nc.scalar.copy(vs[:, lo:TH], vt[:, lo:TH])
                    # in-place product: U tile becomes U*V, then PE
                    # accumulates the W matmul on top (diag cols of U,V are
                    # zero by construction, so skipping them is exact)
                    nc.vector.tensor_tensor(ut[:, lo:TH], ut[:, lo:TH],
                                            vs[:, lo:TH], AL.mult)
                    nc.tensor.matmul(ut, lhs_bt, rt_s["rw_hi"][:, c0:c0 + TH],
                                     start=False, stop=False,
                                     skip_group_check=True)
                    nc.tensor.matmul(ut, lhs_bt, rt_s["rw_lo"][:, c0:c0 + TH],
                                     start=False, stop=True,
                                     skip_group_check=True)
                    # escape PSUM: split 60/40 between ACT and DVE
                    if ncp % 5 < 3:
                        nc.scalar.copy(ot[:, c0:c0 + TH], ut)
                    else:
                        nc.vector.tensor_copy(ot[:, c0:c0 + TH], ut)
                    ncp += 1
                nc.sync.dma_start(
                    out=out_d[bt * P:(bt + 1) * P, w, :], in_=ot)
    nc.finalize()
    return nc


def kernel(x, emb_diag, emb_nondiag):
    key = "prog"
    if key not in _PROGRAM_CACHE:
        _PROGRAM_CACHE[key] = _build_program()
    nc = _PROGRAM_CACHE[key]

    tabs = _host_tables(np.asarray(emb_diag), np.asarray(emb_nondiag))
    tab_names = ("ru_hi", "ru_lo", "rv_hi", "rv_lo", "rw_hi", "rw_lo")
    xf = np.asarray(x).astype(np.float32).astype(ml_dtypes.bfloat16)
    xf = xf.reshape(8, BCORE, R * A)

    in_maps = []
    for core in range(8):
        m = {"xs": xf[core]}
        m.update({nm: t for nm, t in zip(tab_names, tabs)})
        in_maps.append(m)
    res = bass_utils.run_bass_kernel_spmd(nc, in_maps, core_ids=list(range(8)))
    global LAST_RESULT
    LAST_RESULT = res
    outs = [res.results[i]["out"] for i in range(8)]
    return np.concatenate(outs, axis=0)


LAST_RESULT = None


if __name__ == "__main__":
    inputs = {k: np.asarray(v) for k, v in
              np.load("/root/problem/inputs_used.npz").items()}
    out = kernel(**inputs)
    exp = np.load("/root/problem/expected_np.npy")
    err = np.abs(out - exp)
    print("max abs err:", err.max(), "scale-rel:", err.max() / np.abs(exp).max())



# revision 2
# speedup vs baseline: 6.9548x; 6.9548x over previous
"""Trainium2 Bass kernel for nn_CNNEmbedder (surface-code CNN embedder).

The end-to-end call is dominated by the axon device<->host tunnel
(~90 MB/s H2D, ~47 MB/s D2H), so the kernel ships CLASS CODES, not
values: per (batch, window, column) the output value is one of <=9
values selected by (st_i, st_j) in {-1,0,1}^2.  code = st_i + 3*st_j
+ 4 in 0..8 is LINEAR in st, so one K=49 matmul per window computes
byte = code_even + 9*code_odd (<=80, int8-safe) for a PAIR of output
columns: the device writes (B, 23, 588) int8 (55 MB) instead of
(B, 23, 1176) f32 (443 MB).  The host decodes with a single
np.take from a (588*81) u64 LUT (two f32 values per entry).

Device per core (512 batch rows = 4 tiles of 128):
  - int recurrence over windows (gpsimd/vector, bf16), exactly as the
    reference: st,dt in {-1,0,1}
  - per window: PE-transpose st -> lhs[48,512] (+ ones row), one
    [49,128]x[49,294] bf16 matmul per (batch-tile, half), PSUM f32 ->
    int8 pack-copy into an SBUF accumulator, one DMA per batch tile.

Sharding: pure batch data-parallel across 8 cores (512 batch each).
"""
import os
import sys

sys.path.insert(0, "/opt/trn_rl_repo")

import numpy as np
from contextlib import ExitStack

import jax

# Persist compiled executables: the fresh-closure jit inside
# run_bass_kernel_spmd otherwise re-invokes the BIR->NEFF hook on
# every call (~0.65 s) and on every fresh process (~40 s).
jax.config.update("jax_compilation_cache_dir",
                  os.path.expanduser("~/.jax_bass_cache"))
jax.config.update("jax_persistent_cache_min_compile_time_secs", 0)

import concourse.bass as bass
import concourse.tile as tile
from concourse import bacc
from concourse import mybir
from concourse import bass_utils
from concourse.masks import make_identity

F32 = mybir.dt.float32
BF16 = mybir.dt.bfloat16
U8 = mybir.dt.uint8
I8 = mybir.dt.int8
AL = mybir.AluOpType

A = 48            # ancillas
R = 25            # rounds
NW = 23           # windows (R-2)
ND = 1176         # output cols (48 diag + 1128 nondiag)
NPAIR = 1128
NB = ND // 2      # 588 packed bytes per row
HB = NB // 2      # 294, one PSUM-bank half
P = 128
NBT = 4           # batch tiles per core (512 = 4*128)
BCORE = 512       # batch per core
K = 49            # matmul contraction rows: 48 st rows + const row

_PROGRAM_CACHE = {}
_HOST_CONST_CACHE = {}


# ---------------------------------------------------------------- host math
def _pair_list():
    pairs = []
    for iy in range(A):
        for ix in range(iy + 1, A):
            pairs.append((iy, ix))
    return pairs


def _m2_table():
    """(K, NB) bf16-exact coefficients: byte = st @ M2 + 40."""
    M = np.zeros((A, ND), np.float32)
    for a in range(A):
        M[a, a] = 4.0                      # diag col: code = 4*st + 4
    for q, (i, j) in enumerate(_pair_list()):
        M[i, A + q] += 1.0                 # pair col: code = st_i + 3 st_j + 4
        M[j, A + q] += 3.0
    M2 = np.zeros((K, NB), np.float32)
    M2[:A] = M[:, 0::2] + 9.0 * M[:, 1::2]
    M2[K - 1, :] = 40.0                    # 4 + 9*4 (both columns' +4)
    return M2


def _host_lut(emb_diag, emb_nondiag):
    """(NB*81,) u64 LUT: entry[q*81 + v] packs f32 values of columns
    (2q, 2q+1) for byte v = code_even + 9*code_odd."""
    sig_diag = (1.0 / (1.0 + np.exp(-emb_diag[0].astype(np.float64))))
    sg = 1.0 / (1.0 + np.exp(-emb_nondiag[0].astype(np.float64)))      # (1128,4)
    f12 = sg[:, 0]
    f9 = sg[:, 1] * f12
    f8 = sg[:, 2] * f9
    f6 = sg[:, 3] * f8

    # T9[p, code]: output value of column p for code in 0..8
    T9 = np.zeros((ND, 9), np.float32)
    # diag col a: code = 4*(st+1) -> 0:-1 -> 0, 4:0 -> sig, 8:+1 -> 1
    T9[:A, 4] = sig_diag.astype(np.float32)
    T9[:A, 8] = 1.0
    # pair col: code = n_i + 3*n_j, t = (n_i+2)*(n_j+2)
    code = np.arange(9)
    tcode = (code % 3 + 2) * (code // 3 + 2)          # {4,6,8,9,12,16}
    vmap = np.zeros((NPAIR, 17), np.float32)
    vmap[:, 6] = f6
    vmap[:, 8] = f8
    vmap[:, 9] = f9
    vmap[:, 12] = f12
    vmap[:, 16] = 1.0
    T9[A:] = vmap[:, tcode]

    Tb = T9.view(np.uint32)                            # (ND, 9)
    v = np.arange(81)
    lut = (Tb[0::2][:, v % 9].astype(np.uint64)
           | (Tb[1::2][:, v // 9].astype(np.uint64) << np.uint64(32)))
    return np.ascontiguousarray(lut).reshape(-1)       # (NB*81,)


# ---------------------------------------------------------------- program
def _build_program():
    nc = bacc.Bacc(None, target_bir_lowering=False)
    xs_d = nc.declare_dram_parameter("xs", [BCORE, R * A], U8, isOutput=False)
    m2_d = nc.declare_dram_parameter("m2", [K, NB], BF16, isOutput=False)
    out_d = nc.declare_dram_parameter("out", [BCORE, NW, NB], I8, isOutput=True)

    WIDE = NW * A  # 1104

    with ExitStack() as ctx:
        tc = ctx.enter_context(tile.TileContext(nc))
        singles = ctx.enter_context(tc.tile_pool(name="singles", bufs=1))
        wscr = ctx.enter_context(tc.tile_pool(name="wscr", bufs=4))
        sscr = ctx.enter_context(tc.tile_pool(name="sscr", bufs=4))
        pT = ctx.enter_context(tc.tile_pool(name="pT", bufs=2, space="PSUM"))
        pM = ctx.enter_context(tc.tile_pool(name="pM", bufs=3, space="PSUM"))

        ident = singles.tile([P, P], F32)
        make_identity(nc, ident)
        identb = singles.tile([P, P], BF16, tag="identb")
        nc.vector.tensor_copy(identb, ident)

        m2_s = singles.tile([K, NB], BF16, tag="m2")
        nc.sync.dma_start(out=m2_s, in_=m2_d[:, :])

        xts = []
        xbs = []
        for bt in range(NBT):
            xt = singles.tile([P, R * A], U8, tag=f"x{bt}")
            nc.sync.dma_start(out=xt, in_=xs_d[bt * P:(bt + 1) * P, :])
            xts.append(xt)
            xb = singles.tile([P, R * A], BF16, tag=f"xb{bt}")
            nc.gpsimd.tensor_copy(xb, xt)
            xbs.append(xb)

        de_t = singles.tile([P, NBT, WIDE], BF16, tag="de")
        me2_t = singles.tile([P, NBT, WIDE], BF16, tag="me2")
        mep_t = singles.tile([P, NBT, WIDE], BF16, tag="mep")
        one_t = singles.tile([P, WIDE], BF16, tag="one")
        nc.gpsimd.memset(one_t, 1.0)

        # ---- wide precompute (GPSIMD): per b-tile
        for bt in range(NBT):
            xb = xbs[bt]
            a_ap = xb[:, 0:WIDE]
            b_ap = xb[:, A:A + WIDE]
            c_ap = xb[:, 2 * A:2 * A + WIDE]
            t1 = wscr.tile([P, WIDE], BF16, tag="w0")
            d0 = wscr.tile([P, WIDE], BF16, tag="w1")
            w1 = wscr.tile([P, WIDE], BF16, tag="w2")
            u1 = wscr.tile([P, WIDE], BF16, tag="w3")
            u2 = wscr.tile([P, WIDE], BF16, tag="w4")
            nme = wscr.tile([P, WIDE], BF16, tag="w5")
            tmp = wscr.tile([P, WIDE], BF16, tag="w6")
            de1 = wscr.tile([P, WIDE], BF16, tag="w7")
            g = nc.gpsimd
            g.tensor_tensor(t1, a_ap, c_ap, AL.mult)
            g.tensor_tensor(d0, a_ap, c_ap, AL.subtract)
            g.tensor_tensor(de_t[:, bt, :], d0, d0, AL.mult)
            g.tensor_tensor(w1, b_ap, t1, AL.mult)
            g.tensor_tensor(u1, b_ap, t1, AL.add)
            # u2 = u1 - 2*w1
            g.tensor_tensor(tmp, w1, w1, AL.add)
            g.tensor_tensor(u2, u1, tmp, AL.subtract)
            # nme = (de - 1) * u2   ( = -meas_err )
            g.tensor_tensor(de1, de_t[:, bt, :], one_t, AL.subtract)
            g.tensor_tensor(nme, de1, u2, AL.mult)
            # me2 = 1 - 2*me = 2*nme + 1 ; mep = 1 - me = nme + 1
            g.tensor_tensor(tmp, nme, nme, AL.add)
            g.tensor_tensor(me2_t[:, bt, :], tmp, one_t, AL.add)
            g.tensor_tensor(mep_t[:, bt, :], nme, one_t, AL.add)

        st_t = singles.tile([P, NBT, A], BF16, tag="st")
        dt_t = singles.tile([P, NBT, A], BF16, tag="dt")
        nc.vector.memset(st_t, -1.0)
        nc.vector.memset(dt_t, 1.0)

        lhs_t = singles.tile([K, NBT * P], BF16, tag="lhs")
        nc.vector.memset(lhs_t, 1.0)   # row 48 stays 1.0; rows 0:48 overwritten

        obufs = []
        for bt in range(NBT):
            ob = singles.tile([P, NW, 2, HB], I8, tag=f"ob{bt}")
            obufs.append(ob)

        ncp = 0  # pack-copy round robin for ACT/DVE balancing
        for w in range(NW):
            de_w = de_t[:, :, w * A:(w + 1) * A]
            me2_w = me2_t[:, :, w * A:(w + 1) * A]
            mep_w = mep_t[:, :, w * A:(w + 1) * A]
            g = nc.gpsimd
            dt1 = sscr.tile([P, NBT, A], BF16, tag="s0")
            q = sscr.tile([P, NBT, A], BF16, tag="s1")
            s = sscr.tile([P, NBT, A], BF16, tag="s2")
            u2s = sscr.tile([P, NBT, A], BF16, tag="s3")
            wv = sscr.tile([P, NBT, A], BF16, tag="s4")
            z = sscr.tile([P, NBT, A], BF16, tag="s5")
            g.tensor_tensor(dt1, dt_t, me2_w, AL.mult)
            g.tensor_tensor(q, dt1, de_w, AL.mult)
            g.tensor_tensor(s, st_t, q, AL.add)
            nc.vector.tensor_scalar(st_t, s, -1.0, 1.0, AL.max, AL.min)
            g.tensor_tensor(u2s, mep_w, st_t, AL.mult)
            g.tensor_tensor(wv, st_t, dt1, AL.mult)
            nc.vector.scalar_tensor_tensor(z, wv, 1.0, u2s, AL.add, AL.mult)
            g.tensor_tensor(dt_t, dt1, z, AL.subtract)

            # transpose st into lhs rows 0:48 (ones row 48 pre-set)
            pt = pT.tile([A, NBT * P], BF16)
            for bt in range(NBT):
                nc.tensor.transpose(pt[:, bt * P:(bt + 1) * P],
                                    st_t[:, bt, :], identb)
            nc.scalar.copy(lhs_t[0:A, :], pt)

            for bt in range(NBT):
                lhs_bt = lhs_t[:, bt * P:(bt + 1) * P]
                ps = pM.tile([P, 2, 512], F32)
                for ch in range(2):
                    nc.tensor.matmul(ps[:, ch, 0:HB], lhs_bt,
                                     m2_s[:, ch * HB:(ch + 1) * HB],
                                     start=True, stop=True,
                                     skip_group_check=True)
                # pack both halves PSUM f32 -> SBUF int8 in one copy
                if ncp % 5 < 3:
                    nc.scalar.copy(obufs[bt][:, w, :, :], ps[:, :, 0:HB])
                else:
                    nc.vector.tensor_copy(obufs[bt][:, w, :, :], ps[:, :, 0:HB])
                ncp += 1

        for bt in range(NBT):
            nc.sync.dma_start(
                out=out_d[bt * P:(bt + 1) * P, :, :],
                in_=obufs[bt])
    nc.finalize()
    return nc


def kernel(x, emb_diag, emb_nondiag):
    key = "prog"
    if key not in _PROGRAM_CACHE:
        _PROGRAM_CACHE[key] = _build_program()
    nc = _PROGRAM_CACHE[key]

    lut = _host_lut(np.asarray(emb_diag), np.asarray(emb_nondiag))

    if "m2" not in _HOST_CONST_CACHE:
        import ml_dtypes
        _HOST_CONST_CACHE["m2"] = _m2_table().astype(ml_dtypes.bfloat16)
    m2 = _HOST_CONST_CACHE["m2"]
    xu = np.asarray(x).astype(np.uint8).reshape(8, BCORE, R * A)

    in_maps = [{"xs": xu[core], "m2": m2} for core in range(8)]
    res = bass_utils.run_bass_kernel_spmd(nc, in_maps, core_ids=list(range(8)))
    global LAST_RESULT
    LAST_RESULT = res

    # host decode: byte -> two f32 output columns via u64 LUT gather
    offs = (np.arange(NB, dtype=np.int32) * 81)
    out_u64 = np.empty((4096, NW, NB), np.uint64)
    idx = np.empty((BCORE, NW, NB), np.int32)
    for c in range(8):
        pk = res.results[c]["out"]                      # (512, NW, NB) int8
        np.add(pk, offs, out=idx, casting="unsafe")
        np.take(lut, idx, out=out_u64[c * BCORE:(c + 1) * BCORE],
                mode="clip")
    return out_u64.view(np.float32).reshape(4096, NW, ND)


LAST_RESULT = None


if __name__ == "__main__":
    d = np.load("/root/problem/inputs_used.npz")
    inputs = {k: d[k] for k in d.files}
    out = kernel(**inputs)
    exp = np.load("/root/problem/expected_np.npy")
    err = np.abs(out - exp)
    print("max abs err:", err.max(), "scale-rel:", err.max() / np.abs(exp).max())


# revision 3
# speedup vs baseline: 7.6599x; 1.1014x over previous
"""Trainium2 Bass kernel for nn_CNNEmbedder (surface-code CNN embedder).

The end-to-end call is dominated by the axon device<->host tunnel
(~90 MB/s H2D, ~47 MB/s D2H), so the kernel ships CLASS CODES, not
values: per (batch, window, column) the output value is one of <=9
values selected by (st_i, st_j) in {-1,0,1}^2.  code = st_i + 3*st_j
+ 4 in 0..8 is LINEAR in st, so one K=49 matmul per window computes
byte = code_even + 9*code_odd (<=80, int8-safe) for a PAIR of output
columns: the device writes (B, 23, 588) int8 (55 MB) instead of
(B, 23, 1176) f32 (443 MB).  The host decodes with a single
np.take from a (588*81) u64 LUT (two f32 values per entry).

Device per core (512 batch rows = 4 tiles of 128):
  - int recurrence over windows (gpsimd/vector, bf16), exactly as the
    reference: st,dt in {-1,0,1}
  - per window: PE-transpose st -> lhs[48,512] (+ ones row), one
    [49,128]x[49,294] bf16 matmul per (batch-tile, half), PSUM f32 ->
    int8 pack-copy into an SBUF accumulator, one DMA per batch tile.

Sharding: pure batch data-parallel across 8 cores (512 batch each).
"""
import os
import sys

sys.path.insert(0, "/opt/trn_rl_repo")

import numpy as np
from contextlib import ExitStack

import jax

# Persist compiled executables: the fresh-closure jit inside
# run_bass_kernel_spmd otherwise re-invokes the BIR->NEFF hook on
# every call (~0.65 s) and on every fresh process (~40 s).
jax.config.update("jax_compilation_cache_dir",
                  os.path.expanduser("~/.jax_bass_cache"))
jax.config.update("jax_persistent_cache_min_compile_time_secs", 0)

import concourse.bass as bass
import concourse.tile as tile
from concourse import bacc
from concourse import mybir
from concourse import bass_utils
from concourse.masks import make_identity

F32 = mybir.dt.float32
BF16 = mybir.dt.bfloat16
U8 = mybir.dt.uint8
I8 = mybir.dt.int8
AL = mybir.AluOpType

A = 48            # ancillas
R = 25            # rounds
NW = 23           # windows (R-2)
ND = 1176         # output cols (48 diag + 1128 nondiag)
NPAIR = 1128
NB = ND // 2      # 588 packed bytes per row
HB = NB // 2      # 294, one PSUM-bank half
P = 128
NBT = 2           # batch tiles per core (256 = 2*128)
BCORE = 256       # batch per core (half-batch pipelined: 2 SPMD calls)
NHALF = 2         # pipelined halves per kernel() call
K = 49            # matmul contraction rows: 48 st rows + const row

_PROGRAM_CACHE = {}
_HOST_CONST_CACHE = {}


# ---------------------------------------------------------------- host math
def _pair_list():
    pairs = []
    for iy in range(A):
        for ix in range(iy + 1, A):
            pairs.append((iy, ix))
    return pairs


def _m2_table():
    """(K, NB) bf16-exact coefficients: byte = st @ M2 + 40."""
    M = np.zeros((A, ND), np.float32)
    for a in range(A):
        M[a, a] = 4.0                      # diag col: code = 4*st + 4
    for q, (i, j) in enumerate(_pair_list()):
        M[i, A + q] += 1.0                 # pair col: code = st_i + 3 st_j + 4
        M[j, A + q] += 3.0
    M2 = np.zeros((K, NB), np.float32)
    M2[:A] = M[:, 0::2] + 9.0 * M[:, 1::2]
    M2[K - 1, :] = 40.0                    # 4 + 9*4 (both columns' +4)
    return M2


def _host_lut(emb_diag, emb_nondiag):
    """(NB*81,) u64 LUT: entry[q*81 + v] packs f32 values of columns
    (2q, 2q+1) for byte v = code_even + 9*code_odd."""
    sig_diag = (1.0 / (1.0 + np.exp(-emb_diag[0].astype(np.float64))))
    sg = 1.0 / (1.0 + np.exp(-emb_nondiag[0].astype(np.float64)))      # (1128,4)
    f12 = sg[:, 0]
    f9 = sg[:, 1] * f12
    f8 = sg[:, 2] * f9
    f6 = sg[:, 3] * f8

    # T9[p, code]: output value of column p for code in 0..8
    T9 = np.zeros((ND, 9), np.float32)
    # diag col a: code = 4*(st+1) -> 0:-1 -> 0, 4:0 -> sig, 8:+1 -> 1
    T9[:A, 4] = sig_diag.astype(np.float32)
    T9[:A, 8] = 1.0
    # pair col: code = n_i + 3*n_j, t = (n_i+2)*(n_j+2)
    code = np.arange(9)
    tcode = (code % 3 + 2) * (code // 3 + 2)          # {4,6,8,9,12,16}
    vmap = np.zeros((NPAIR, 17), np.float32)
    vmap[:, 6] = f6
    vmap[:, 8] = f8
    vmap[:, 9] = f9
    vmap[:, 12] = f12
    vmap[:, 16] = 1.0
    T9[A:] = vmap[:, tcode]

    Tb = T9.view(np.uint32)                            # (ND, 9)
    v = np.arange(81)
    lut = (Tb[0::2][:, v % 9].astype(np.uint64)
           | (Tb[1::2][:, v // 9].astype(np.uint64) << np.uint64(32)))
    return np.ascontiguousarray(lut).reshape(-1)       # (NB*81,)


# ---------------------------------------------------------------- program
def _build_program():
    nc = bacc.Bacc(None, target_bir_lowering=False)
    xs_d = nc.declare_dram_parameter("xs", [BCORE, R * A], U8, isOutput=False)
    m2_d = nc.declare_dram_parameter("m2", [K, NB], BF16, isOutput=False)
    out_d = nc.declare_dram_parameter("out", [BCORE, NW, NB], I8, isOutput=True)

    WIDE = NW * A  # 1104

    with ExitStack() as ctx:
        tc = ctx.enter_context(tile.TileContext(nc))
        singles = ctx.enter_context(tc.tile_pool(name="singles", bufs=1))
        wscr = ctx.enter_context(tc.tile_pool(name="wscr", bufs=4))
        sscr = ctx.enter_context(tc.tile_pool(name="sscr", bufs=4))
        pT = ctx.enter_context(tc.tile_pool(name="pT", bufs=2, space="PSUM"))
        pM = ctx.enter_context(tc.tile_pool(name="pM", bufs=3, space="PSUM"))

        ident = singles.tile([P, P], F32)
        make_identity(nc, ident)
        identb = singles.tile([P, P], BF16, tag="identb")
        nc.vector.tensor_copy(identb, ident)

        m2_s = singles.tile([K, NB], BF16, tag="m2")
        nc.sync.dma_start(out=m2_s, in_=m2_d[:, :])

        xts = []
        xbs = []
        for bt in range(NBT):
            xt = singles.tile([P, R * A], U8, tag=f"x{bt}")
            nc.sync.dma_start(out=xt, in_=xs_d[bt * P:(bt + 1) * P, :])
            xts.append(xt)
            xb = singles.tile([P, R * A], BF16, tag=f"xb{bt}")
            nc.gpsimd.tensor_copy(xb, xt)
            xbs.append(xb)

        de_t = singles.tile([P, NBT, WIDE], BF16, tag="de")
        me2_t = singles.tile([P, NBT, WIDE], BF16, tag="me2")
        mep_t = singles.tile([P, NBT, WIDE], BF16, tag="mep")
        one_t = singles.tile([P, WIDE], BF16, tag="one")
        nc.gpsimd.memset(one_t, 1.0)

        # ---- wide precompute (GPSIMD): per b-tile
        for bt in range(NBT):
            xb = xbs[bt]
            a_ap = xb[:, 0:WIDE]
            b_ap = xb[:, A:A + WIDE]
            c_ap = xb[:, 2 * A:2 * A + WIDE]
            t1 = wscr.tile([P, WIDE], BF16, tag="w0")
            d0 = wscr.tile([P, WIDE], BF16, tag="w1")
            w1 = wscr.tile([P, WIDE], BF16, tag="w2")
            u1 = wscr.tile([P, WIDE], BF16, tag="w3")
            u2 = wscr.tile([P, WIDE], BF16, tag="w4")
            nme = wscr.tile([P, WIDE], BF16, tag="w5")
            tmp = wscr.tile([P, WIDE], BF16, tag="w6")
            de1 = wscr.tile([P, WIDE], BF16, tag="w7")
            g = nc.gpsimd
            g.tensor_tensor(t1, a_ap, c_ap, AL.mult)
            g.tensor_tensor(d0, a_ap, c_ap, AL.subtract)
            g.tensor_tensor(de_t[:, bt, :], d0, d0, AL.mult)
            g.tensor_tensor(w1, b_ap, t1, AL.mult)
            g.tensor_tensor(u1, b_ap, t1, AL.add)
            # u2 = u1 - 2*w1
            g.tensor_tensor(tmp, w1, w1, AL.add)
            g.tensor_tensor(u2, u1, tmp, AL.subtract)
            # nme = (de - 1) * u2   ( = -meas_err )
            g.tensor_tensor(de1, de_t[:, bt, :], one_t, AL.subtract)
            g.tensor_tensor(nme, de1, u2, AL.mult)
            # me2 = 1 - 2*me = 2*nme + 1 ; mep = 1 - me = nme + 1
            g.tensor_tensor(tmp, nme, nme, AL.add)
            g.tensor_tensor(me2_t[:, bt, :], tmp, one_t, AL.add)
            g.tensor_tensor(mep_t[:, bt, :], nme, one_t, AL.add)

        st_t = singles.tile([P, NBT, A], BF16, tag="st")
        dt_t = singles.tile([P, NBT, A], BF16, tag="dt")
        nc.vector.memset(st_t, -1.0)
        nc.vector.memset(dt_t, 1.0)

        lhs_t = singles.tile([K, NBT * P], BF16, tag="lhs")
        nc.vector.memset(lhs_t, 1.0)   # row 48 stays 1.0; rows 0:48 overwritten

        obufs = []
        for bt in range(NBT):
            ob = singles.tile([P, NW, 2, HB], I8, tag=f"ob{bt}")
            obufs.append(ob)

        ncp = 0  # pack-copy round robin for ACT/DVE balancing
        for w in range(NW):
            de_w = de_t[:, :, w * A:(w + 1) * A]
            me2_w = me2_t[:, :, w * A:(w + 1) * A]
            mep_w = mep_t[:, :, w * A:(w + 1) * A]
            g = nc.gpsimd
            dt1 = sscr.tile([P, NBT, A], BF16, tag="s0")
            q = sscr.tile([P, NBT, A], BF16, tag="s1")
            s = sscr.tile([P, NBT, A], BF16, tag="s2")
            u2s = sscr.tile([P, NBT, A], BF16, tag="s3")
            wv = sscr.tile([P, NBT, A], BF16, tag="s4")
            z = sscr.tile([P, NBT, A], BF16, tag="s5")
            g.tensor_tensor(dt1, dt_t, me2_w, AL.mult)
            g.tensor_tensor(q, dt1, de_w, AL.mult)
            g.tensor_tensor(s, st_t, q, AL.add)
            nc.vector.tensor_scalar(st_t, s, -1.0, 1.0, AL.max, AL.min)
            g.tensor_tensor(u2s, mep_w, st_t, AL.mult)
            g.tensor_tensor(wv, st_t, dt1, AL.mult)
            nc.vector.scalar_tensor_tensor(z, wv, 1.0, u2s, AL.add, AL.mult)
            g.tensor_tensor(dt_t, dt1, z, AL.subtract)

            # transpose st into lhs rows 0:48 (ones row 48 pre-set)
            pt = pT.tile([A, NBT * P], BF16)
            for bt in range(NBT):
                nc.tensor.transpose(pt[:, bt * P:(bt + 1) * P],
                                    st_t[:, bt, :], identb)
            nc.scalar.copy(lhs_t[0:A, :], pt)

            for bt in range(NBT):
                lhs_bt = lhs_t[:, bt * P:(bt + 1) * P]
                ps = pM.tile([P, 2, 512], F32)
                for ch in range(2):
                    nc.tensor.matmul(ps[:, ch, 0:HB], lhs_bt,
                                     m2_s[:, ch * HB:(ch + 1) * HB],
                                     start=True, stop=True,
                                     skip_group_check=True)
                # pack both halves PSUM f32 -> SBUF int8 in one copy
                if ncp % 5 < 3:
                    nc.scalar.copy(obufs[bt][:, w, :, :], ps[:, :, 0:HB])
                else:
                    nc.vector.tensor_copy(obufs[bt][:, w, :, :], ps[:, :, 0:HB])
                ncp += 1

        for bt in range(NBT):
            nc.sync.dma_start(
                out=out_d[bt * P:(bt + 1) * P, :, :],
                in_=obufs[bt])
    nc.finalize()
    return nc


_POOL = None
_WARMED = False


def _decode_half(res, out_u64, h, lut, offs, idx):
    base = h * 8 * BCORE
    for c in range(8):
        pk = res.results[c]["out"]                      # (BCORE, NW, NB) int8
        np.add(pk, offs, out=idx, casting="unsafe")
        np.take(lut, idx,
                out=out_u64[base + c * BCORE:base + (c + 1) * BCORE],
                mode="clip")


def kernel(x, emb_diag, emb_nondiag):
    global LAST_RESULT, _POOL, _WARMED
    key = "prog"
    if key not in _PROGRAM_CACHE:
        _PROGRAM_CACHE[key] = _build_program()
    nc = _PROGRAM_CACHE[key]
    if _POOL is None:
        from concurrent.futures import ThreadPoolExecutor
        _POOL = ThreadPoolExecutor(NHALF)

    lut = _host_lut(np.asarray(emb_diag), np.asarray(emb_nondiag))

    if "m2" not in _HOST_CONST_CACHE:
        import ml_dtypes
        _HOST_CONST_CACHE["m2"] = _m2_table().astype(ml_dtypes.bfloat16)
    m2 = _HOST_CONST_CACHE["m2"]
    xu = np.asarray(x).astype(np.uint8).reshape(NHALF, 8, BCORE, R * A)

    def run_half(h):
        in_maps = [{"xs": xu[h, c], "m2": m2} for c in range(8)]
        return bass_utils.run_bass_kernel_spmd(nc, in_maps,
                                               core_ids=list(range(8)))

    offs = (np.arange(NB, dtype=np.int32) * 81)
    out_u64 = np.empty((4096, NW, NB), np.uint64)
    idx = np.empty((BCORE, NW, NB), np.int32)

    if not _WARMED:
        # first call in the process: serialize (avoid two concurrent
        # NEFF compiles racing on one CPU); populates jit/NEFF caches
        results = [run_half(h) for h in range(NHALF)]
        for h, res in enumerate(results):
            _decode_half(res, out_u64, h, lut, offs, idx)
        LAST_RESULT = results[-1]
        _WARMED = True
        return out_u64.view(np.float32).reshape(4096, NW, ND)

    # warm path: both halves in flight; decode half h while half h+1
    # downloads (tunnel is ~full-duplex, np.take releases the GIL)
    futs = [_POOL.submit(run_half, h) for h in range(NHALF)]
    for h, f in enumerate(futs):
        res = f.result()
        _decode_half(res, out_u64, h, lut, offs, idx)
        LAST_RESULT = res
    return out_u64.view(np.float32).reshape(4096, NW, ND)


LAST_RESULT = None


if __name__ == "__main__":
    d = np.load("/root/problem/inputs_used.npz")
    inputs = {k: d[k] for k in d.files}
    out = kernel(**inputs)
    exp = np.load("/root/problem/expected_np.npy")
    err = np.abs(out - exp)
    print("max abs err:", err.max(), "scale-rel:", err.max() / np.abs(exp).max())


# revision 4
# speedup vs baseline: 8.0843x; 1.0554x over previous
"""Trainium2 Bass kernel for nn_CNNEmbedder (surface-code CNN embedder).

The end-to-end call is dominated by the axon device<->host tunnel
(~90 MB/s H2D, ~47 MB/s D2H), so the kernel ships CLASS CODES, not
values: per (batch, window, column) the output value is one of <=9
values selected by (st_i, st_j) in {-1,0,1}^2.  code = st_i + 3*st_j
+ 4 in 0..8 is LINEAR in st, so one K=49 matmul per window computes
byte = code_even + 9*code_odd (<=80, int8-safe) for a PAIR of output
columns: the device writes (B, 23, 588) int8 (55 MB) instead of
(B, 23, 1176) f32 (443 MB).  The host decodes with a single
np.take from a (588*81) u64 LUT (two f32 values per entry).

Device per core (512 batch rows = 4 tiles of 128):
  - int recurrence over windows (gpsimd/vector, bf16), exactly as the
    reference: st,dt in {-1,0,1}
  - per window: PE-transpose st -> lhs[48,512] (+ ones row), one
    [49,128]x[49,294] bf16 matmul per (batch-tile, half), PSUM f32 ->
    int8 pack-copy into an SBUF accumulator, one DMA per batch tile.

Sharding: pure batch data-parallel across 8 cores (512 batch each).
"""
import os
import sys

sys.path.insert(0, "/opt/trn_rl_repo")

import numpy as np
from contextlib import ExitStack

import jax

# Persist compiled executables: the fresh-closure jit inside
# run_bass_kernel_spmd otherwise re-invokes the BIR->NEFF hook on
# every call (~0.65 s) and on every fresh process (~40 s).
jax.config.update("jax_compilation_cache_dir",
                  os.path.expanduser("~/.jax_bass_cache"))
jax.config.update("jax_persistent_cache_min_compile_time_secs", 0)

import concourse.bass as bass
import concourse.tile as tile
from concourse import bacc
from concourse import mybir
from concourse import bass_utils
from concourse.masks import make_identity

F32 = mybir.dt.float32
BF16 = mybir.dt.bfloat16
U8 = mybir.dt.uint8
I8 = mybir.dt.int8
AL = mybir.AluOpType

A = 48            # ancillas
R = 25            # rounds
NW = 23           # windows (R-2)
ND = 1176         # output cols (48 diag + 1128 nondiag)
NPAIR = 1128
NB = ND // 2      # 588 packed bytes per row
HB = NB // 2      # 294, one PSUM-bank half
P = 128
NBT = 2           # batch tiles per core (256 = 2*128)
BCORE = 256       # batch per core (half-batch pipelined: 2 SPMD calls)
NHALF = 2         # pipelined halves per kernel() call
K = 49            # matmul contraction rows: 48 st rows + const row

_PROGRAM_CACHE = {}
_HOST_CONST_CACHE = {}


# ---------------------------------------------------------------- host math
def _pair_list():
    pairs = []
    for iy in range(A):
        for ix in range(iy + 1, A):
            pairs.append((iy, ix))
    return pairs


def _m2_table():
    """(K, NB) bf16-exact coefficients: byte = st @ M2 + 40."""
    M = np.zeros((A, ND), np.float32)
    for a in range(A):
        M[a, a] = 4.0                      # diag col: code = 4*st + 4
    for q, (i, j) in enumerate(_pair_list()):
        M[i, A + q] += 1.0                 # pair col: code = st_i + 3 st_j + 4
        M[j, A + q] += 3.0
    M2 = np.zeros((K, NB), np.float32)
    M2[:A] = M[:, 0::2] + 9.0 * M[:, 1::2]
    M2[K - 1, :] = 40.0                    # 4 + 9*4 (both columns' +4)
    return M2


def _host_lut(emb_diag, emb_nondiag):
    """(NB*81,) u64 LUT: entry[q*81 + v] packs f32 values of columns
    (2q, 2q+1) for byte v = code_even + 9*code_odd."""
    sig_diag = (1.0 / (1.0 + np.exp(-emb_diag[0].astype(np.float64))))
    sg = 1.0 / (1.0 + np.exp(-emb_nondiag[0].astype(np.float64)))      # (1128,4)
    f12 = sg[:, 0]
    f9 = sg[:, 1] * f12
    f8 = sg[:, 2] * f9
    f6 = sg[:, 3] * f8

    # T9[p, code]: output value of column p for code in 0..8
    T9 = np.zeros((ND, 9), np.float32)
    # diag col a: code = 4*(st+1) -> 0:-1 -> 0, 4:0 -> sig, 8:+1 -> 1
    T9[:A, 4] = sig_diag.astype(np.float32)
    T9[:A, 8] = 1.0
    # pair col: code = n_i + 3*n_j, t = (n_i+2)*(n_j+2)
    code = np.arange(9)
    tcode = (code % 3 + 2) * (code // 3 + 2)          # {4,6,8,9,12,16}
    vmap = np.zeros((NPAIR, 17), np.float32)
    vmap[:, 6] = f6
    vmap[:, 8] = f8
    vmap[:, 9] = f9
    vmap[:, 12] = f12
    vmap[:, 16] = 1.0
    T9[A:] = vmap[:, tcode]

    Tb = T9.view(np.uint32)                            # (ND, 9)
    v = np.arange(81)
    lut = (Tb[0::2][:, v % 9].astype(np.uint64)
           | (Tb[1::2][:, v // 9].astype(np.uint64) << np.uint64(32)))
    return np.ascontiguousarray(lut).reshape(-1)       # (NB*81,)


# ---------------------------------------------------------------- program
def _build_program():
    nc = bacc.Bacc(None, target_bir_lowering=False)
    xs_d = nc.declare_dram_parameter("xs", [BCORE, R * A], U8, isOutput=False)
    m2_d = nc.declare_dram_parameter("m2", [K, NB], BF16, isOutput=False)
    out_d = nc.declare_dram_parameter("out", [BCORE, NW, NB], I8, isOutput=True)

    WIDE = NW * A  # 1104

    with ExitStack() as ctx:
        tc = ctx.enter_context(tile.TileContext(nc))
        singles = ctx.enter_context(tc.tile_pool(name="singles", bufs=1))
        wscr = ctx.enter_context(tc.tile_pool(name="wscr", bufs=4))
        sscr = ctx.enter_context(tc.tile_pool(name="sscr", bufs=4))
        pT = ctx.enter_context(tc.tile_pool(name="pT", bufs=2, space="PSUM"))
        pM = ctx.enter_context(tc.tile_pool(name="pM", bufs=3, space="PSUM"))

        ident = singles.tile([P, P], F32)
        make_identity(nc, ident)
        identb = singles.tile([P, P], BF16, tag="identb")
        nc.vector.tensor_copy(identb, ident)

        m2_s = singles.tile([K, NB], BF16, tag="m2")
        nc.sync.dma_start(out=m2_s, in_=m2_d[:, :])

        xts = []
        xbs = []
        for bt in range(NBT):
            xt = singles.tile([P, R * A], U8, tag=f"x{bt}")
            nc.sync.dma_start(out=xt, in_=xs_d[bt * P:(bt + 1) * P, :])
            xts.append(xt)
            xb = singles.tile([P, R * A], BF16, tag=f"xb{bt}")
            nc.gpsimd.tensor_copy(xb, xt)
            xbs.append(xb)

        de_t = singles.tile([P, NBT, WIDE], BF16, tag="de")
        me2_t = singles.tile([P, NBT, WIDE], BF16, tag="me2")
        mep_t = singles.tile([P, NBT, WIDE], BF16, tag="mep")
        one_t = singles.tile([P, WIDE], BF16, tag="one")
        nc.gpsimd.memset(one_t, 1.0)

        # ---- wide precompute (GPSIMD): per b-tile
        for bt in range(NBT):
            xb = xbs[bt]
            a_ap = xb[:, 0:WIDE]
            b_ap = xb[:, A:A + WIDE]
            c_ap = xb[:, 2 * A:2 * A + WIDE]
            t1 = wscr.tile([P, WIDE], BF16, tag="w0")
            d0 = wscr.tile([P, WIDE], BF16, tag="w1")
            w1 = wscr.tile([P, WIDE], BF16, tag="w2")
            u1 = wscr.tile([P, WIDE], BF16, tag="w3")
            u2 = wscr.tile([P, WIDE], BF16, tag="w4")
            nme = wscr.tile([P, WIDE], BF16, tag="w5")
            tmp = wscr.tile([P, WIDE], BF16, tag="w6")
            de1 = wscr.tile([P, WIDE], BF16, tag="w7")
            g = nc.gpsimd
            g.tensor_tensor(t1, a_ap, c_ap, AL.mult)
            g.tensor_tensor(d0, a_ap, c_ap, AL.subtract)
            g.tensor_tensor(de_t[:, bt, :], d0, d0, AL.mult)
            g.tensor_tensor(w1, b_ap, t1, AL.mult)
            g.tensor_tensor(u1, b_ap, t1, AL.add)
            # u2 = u1 - 2*w1
            g.tensor_tensor(tmp, w1, w1, AL.add)
            g.tensor_tensor(u2, u1, tmp, AL.subtract)
            # nme = (de - 1) * u2   ( = -meas_err )
            g.tensor_tensor(de1, de_t[:, bt, :], one_t, AL.subtract)
            g.tensor_tensor(nme, de1, u2, AL.mult)
            # me2 = 1 - 2*me = 2*nme + 1 ; mep = 1 - me = nme + 1
            g.tensor_tensor(tmp, nme, nme, AL.add)
            g.tensor_tensor(me2_t[:, bt, :], tmp, one_t, AL.add)
            g.tensor_tensor(mep_t[:, bt, :], nme, one_t, AL.add)

        st_t = singles.tile([P, NBT, A], BF16, tag="st")
        dt_t = singles.tile([P, NBT, A], BF16, tag="dt")
        nc.vector.memset(st_t, -1.0)
        nc.vector.memset(dt_t, 1.0)

        lhs_t = singles.tile([K, NBT * P], BF16, tag="lhs")
        nc.vector.memset(lhs_t, 1.0)   # row 48 stays 1.0; rows 0:48 overwritten

        obufs = []
        for bt in range(NBT):
            ob = singles.tile([P, NW, 2, HB], I8, tag=f"ob{bt}")
            obufs.append(ob)

        ncp = 0  # pack-copy round robin for ACT/DVE balancing
        for w in range(NW):
            de_w = de_t[:, :, w * A:(w + 1) * A]
            me2_w = me2_t[:, :, w * A:(w + 1) * A]
            mep_w = mep_t[:, :, w * A:(w + 1) * A]
            g = nc.gpsimd
            dt1 = sscr.tile([P, NBT, A], BF16, tag="s0")
            q = sscr.tile([P, NBT, A], BF16, tag="s1")
            s = sscr.tile([P, NBT, A], BF16, tag="s2")
            u2s = sscr.tile([P, NBT, A], BF16, tag="s3")
            wv = sscr.tile([P, NBT, A], BF16, tag="s4")
            z = sscr.tile([P, NBT, A], BF16, tag="s5")
            g.tensor_tensor(dt1, dt_t, me2_w, AL.mult)
            g.tensor_tensor(q, dt1, de_w, AL.mult)
            g.tensor_tensor(s, st_t, q, AL.add)
            nc.vector.tensor_scalar(st_t, s, -1.0, 1.0, AL.max, AL.min)
            g.tensor_tensor(u2s, mep_w, st_t, AL.mult)
            g.tensor_tensor(wv, st_t, dt1, AL.mult)
            nc.vector.scalar_tensor_tensor(z, wv, 1.0, u2s, AL.add, AL.mult)
            g.tensor_tensor(dt_t, dt1, z, AL.subtract)

            # transpose st into lhs rows 0:48 (ones row 48 pre-set)
            pt = pT.tile([A, NBT * P], BF16)
            for bt in range(NBT):
                nc.tensor.transpose(pt[:, bt * P:(bt + 1) * P],
                                    st_t[:, bt, :], identb)
            nc.scalar.copy(lhs_t[0:A, :], pt)

            for bt in range(NBT):
                lhs_bt = lhs_t[:, bt * P:(bt + 1) * P]
                ps = pM.tile([P, 2, 512], F32)
                for ch in range(2):
                    nc.tensor.matmul(ps[:, ch, 0:HB], lhs_bt,
                                     m2_s[:, ch * HB:(ch + 1) * HB],
                                     start=True, stop=True,
                                     skip_group_check=True)
                # pack both halves PSUM f32 -> SBUF int8 in one copy
                if ncp % 5 < 3:
                    nc.scalar.copy(obufs[bt][:, w, :, :], ps[:, :, 0:HB])
                else:
                    nc.vector.tensor_copy(obufs[bt][:, w, :, :], ps[:, :, 0:HB])
                ncp += 1

        for bt in range(NBT):
            nc.sync.dma_start(
                out=out_d[bt * P:(bt + 1) * P, :, :],
                in_=obufs[bt])
    nc.finalize()
    return nc


_POOL = None
_WARMED = False


def _decode_half(res, out_u64, h, lut, offs, idx):
    base = h * 8 * BCORE
    for c in range(8):
        pk = res.results[c]["out"]                      # (BCORE, NW, NB) int8
        np.add(pk, offs, out=idx, casting="unsafe")
        np.take(lut, idx,
                out=out_u64[base + c * BCORE:base + (c + 1) * BCORE],
                mode="clip")


def kernel(x, emb_diag, emb_nondiag):
    global LAST_RESULT, _POOL, _WARMED
    key = "prog"
    if key not in _PROGRAM_CACHE:
        _PROGRAM_CACHE[key] = _build_program()
    nc = _PROGRAM_CACHE[key]
    if _POOL is None:
        from concurrent.futures import ThreadPoolExecutor
        _POOL = ThreadPoolExecutor(NHALF)

    lut = _host_lut(np.asarray(emb_diag), np.asarray(emb_nondiag))

    if "m2" not in _HOST_CONST_CACHE:
        import ml_dtypes
        _HOST_CONST_CACHE["m2"] = _m2_table().astype(ml_dtypes.bfloat16)
    m2 = _HOST_CONST_CACHE["m2"]
    xu = np.asarray(x).astype(np.uint8).reshape(NHALF, 8, BCORE, R * A)

    def run_half(h):
        in_maps = [{"xs": xu[h, c], "m2": m2} for c in range(8)]
        return bass_utils.run_bass_kernel_spmd(nc, in_maps,
                                               core_ids=list(range(8)))

    offs = (np.arange(NB, dtype=np.int32) * 81)
    out_u64 = np.empty((4096, NW, NB), np.uint64)
    idx = np.empty((BCORE, NW, NB), np.int32)

    if not _WARMED:
        # first call in the process: serialize (avoid two concurrent
        # NEFF compiles racing on one CPU); populates jit/NEFF caches
        results = [run_half(h) for h in range(NHALF)]
        for h, res in enumerate(results):
            _decode_half(res, out_u64, h, lut, offs, idx)
        LAST_RESULT = results[-1]
        _WARMED = True
        return out_u64.view(np.float32).reshape(4096, NW, ND)

    # warm path: both halves in flight; decode half h while half h+1
    # downloads (tunnel is ~full-duplex, np.take releases the GIL).
    # Stagger the second submit so its upload overlaps the first
    # half's download instead of queueing ahead of it.
    import time as _time
    futs = [_POOL.submit(run_half, 0)]
    _time.sleep(0.3)
    futs.append(_POOL.submit(run_half, 1))
    for h, f in enumerate(futs):
        res = f.result()
        _decode_half(res, out_u64, h, lut, offs, idx)
        LAST_RESULT = res
    return out_u64.view(np.float32).reshape(4096, NW, ND)


LAST_RESULT = None


if __name__ == "__main__":
    d = np.load("/root/problem/inputs_used.npz")
    inputs = {k: d[k] for k in d.files}
    out = kernel(**inputs)
    exp = np.load("/root/problem/expected_np.npy")
    err = np.abs(out - exp)
    print("max abs err:", err.max(), "scale-rel:", err.max() / np.abs(exp).max())
